# revision 1
# baseline (speedup 1.0000x reference)
"""Trainium2 Bass kernel for causal multi-head attention with LoRA (QKV + proj).

Problem (hardcoded): B=4, T=2048, C=1024, NH=16, HD=64, RANK=56, alpha=8.

Sharding (2D): 8 cores = 4 batches x 2 head-groups. Each core owns one batch
(2048 tokens) and 8 heads (4 head-pairs). QKV weights are column-sliced per
head-group (12 chunks of 128 output dims + shared LoRA-A); the output
projection is row-parallel over the core's 512 y dims, and the two cores of a
batch have their partial outputs summed on the host (LoRA-proj partials sum
the same way since B@(r0+r1) = B@r0 + B@r1).

All matmuls run fp16 (fp32 accumulation in PSUM). Layout is transposed:
activations are [feature(partition), token(free)].

Key structure vs a naive port:
- QK is row-packed: the two heads of a pair run as concurrent K=64 matmuls
  at PE tile positions (0,0)/(64,0) (auto-derived from base partitions), so
  one pass of the moving operand scores both heads.
- The two heads' score tiles live in one [128,2,512] PSUM tile (2 adjacent
  banks) so a single ACT exp instruction covers both (halves ACT op count).
- V is transposed to token-major per 128-block on the PE; head0's copy goes
  into vaA ([64 v | ones] stride 66, z lands at PSUM row 64) and head1's into
  vaB (ones-first stride 130: [1 ones | 63 zero | 64 v]), so head1's PV
  output lands y at PSUM rows 64:128 and z at row 0 - no partition-shift DMA
  is needed to assemble yt, and softmax-z normalization chains are short:
  copy z row -> (DMA hop to partition 0 for h0) -> reciprocal -> gpsimd
  partition_broadcast -> one multiply into yt.
- QKV bias is folded into row 56 of the LoRA-B stationary (r row 56 is set
  to 1.0), so all PSUM->SBUF drains are plain copies and can be spread over
  the vector and scalar engines (the scalar engine is reserved mostly for
  exp, which paces attention otherwise).
"""
import os
import sys
import numpy as np

if "/opt/trn_rl_repo" not in sys.path:
    sys.path.insert(0, "/opt/trn_rl_repo")

import concourse.bass as bass  # noqa: E402
from concourse import bacc  # noqa: E402
import concourse.mybir as mybir  # noqa: E402
import concourse.tile as tile  # noqa: E402
from concourse.bass_utils import run_bass_kernel_spmd  # noqa: E402

B, T, C = 4, 2048, 1024
NH, HD, RANK = 16, 64, 56
SCALING = 8.0 / 56.0
NCORES = 8
TOK = 512             # token chunk (matmul moving dim)
NT4 = T // TOK        # 4 token chunks
NCIN = C // 128       # 8 input-feature chunks
HPG = 4               # head-pairs per core (8 heads)
NCH = 12              # qkv output chunks per core (3 types x 4 head-pairs)
VA_A_W = 16 * 66 + 64     # 1120
VA_B_W = 16 * 130         # 2080
F32 = mybir.dt.float32
F16 = mybir.dt.float16
EXPF = mybir.ActivationFunctionType.Exp

_cache = {}


def _build():
    nc = bacc.Bacc("TRN2", target_bir_lowering=False, debug=False,
                   num_devices=NCORES)
    xT = nc.dram_tensor("xT", [C, T], F16, kind="ExternalInput")
    Wq = nc.dram_tensor("Wq", [128, NCIN, NCH * 128], F16,
                        kind="ExternalInput")
    Aq = nc.dram_tensor("Aq", [128, NCIN, 128], F16, kind="ExternalInput")
    Bq = nc.dram_tensor("Bq", [128, NCH * 128], F16, kind="ExternalInput")
    Wp = nc.dram_tensor("Wp", [128, HPG, C], F16, kind="ExternalInput")
    Ap = nc.dram_tensor("Ap", [128, HPG, 128], F16, kind="ExternalInput")
    Bp = nc.dram_tensor("Bp", [128, C], F16, kind="ExternalInput")
    tri = nc.dram_tensor("tri", [128, 128], F16, kind="ExternalInput")
    onesb = nc.dram_tensor("onesb", [128, 16], F16, kind="ExternalInput")
    eye = nc.dram_tensor("eye", [128, 128], F16, kind="ExternalInput")
    outT = nc.dram_tensor("outT", [C, T], F16, kind="ExternalOutput")
    DBG = os.environ.get("KDBG") == "1"
    if DBG:
        dq4 = nc.dram_tensor("dq4", [128, HPG, T], F16, kind="ExternalOutput")
        dk4 = nc.dram_tensor("dk4", [128, HPG, T], F16, kind="ExternalOutput")
        dv4 = nc.dram_tensor("dv4", [128, HPG, T], F16, kind="ExternalOutput")
        dy4 = nc.dram_tensor("dy4", [128, HPG, T], F16, kind="ExternalOutput")
        dvaA = nc.dram_tensor("dvaA", [128, VA_A_W], F16,
                              kind="ExternalOutput")
        dvaB = nc.dram_tensor("dvaB", [128, VA_B_W], F16,
                              kind="ExternalOutput")
        des = [nc.dram_tensor(f"de{j}", [128, 2, TOK], F16,
                              kind="ExternalOutput") for j in range(4)]

    with tile.TileContext(nc) as tc:
        with (
            tc.tile_pool(name="consts", bufs=1) as consts,
            tc.tile_pool(name="persist", bufs=1) as persist,
            tc.tile_pool(name="xtp", bufs=16) as xtp,
            tc.tile_pool(name="expp", bufs=12) as expp,
            tc.tile_pool(name="small", bufs=2) as small,
            tc.tile_pool(name="ps", bufs=1, space="PSUM") as ps,
        ):
            aq_sb = consts.tile([128, NCIN, 128], F16)
            nc.sync.dma_start(aq_sb[:], Aq[:])
            wq_sb = consts.tile([128, NCIN, NCH * 128], F16)
            lb_sb = consts.tile([128, NCH * 128], F16)
            nc.sync.dma_start(lb_sb[:], Bq[:])
            eye_sb = consts.tile([128, 128], F16)
            nc.sync.dma_start(eye_sb[:], eye[:])
            tri_sb = consts.tile([128, 128], F16)
            nc.sync.dma_start(tri_sb[:], tri[:])
            wp_sb = consts.tile([128, HPG, C], F16)
            ap_sb = consts.tile([128, HPG, 128], F16)
            pb_sb = consts.tile([128, C], F16)

            # persistent activations: [128, hp, T] views per head-pair
            q4 = persist.tile([128, HPG, T], F16, tag="q4")
            k4 = persist.tile([128, HPG, T], F16, tag="k4")
            v4 = persist.tile([128, HPG, T], F16, tag="v4")
            y4 = persist.tile([128, HPG, T], F16, tag="y4")
            vaA = [persist.tile([128, VA_A_W], F16, tag=f"vaA{hp}",
                                name=f"vaA{hp}") for hp in range(HPG)]
            vaB = [persist.tile([128, VA_B_W], F16, tag=f"vaB{hp}",
                                name=f"vaB{hp}") for hp in range(HPG)]

            # va init: zero-fill, then strided ones columns
            for hp in range(HPG):
                nc.gpsimd.memset(vaA[hp][:], 0.0)
                nc.gpsimd.memset(vaB[hp][:], 0.0)
                vaAv = vaA[hp][:, 0:16 * 66].rearrange(
                    "p (j c) -> p j c", c=66)
                nc.vector.memset(vaAv[:, :, 64:65], 1.0)
                vaBv = vaB[hp][:, 0:16 * 130].rearrange(
                    "p (j c) -> p j c", c=130)
                nc.vector.memset(vaBv[:, :, 0:1], 1.0)

            def load_xts(t4):
                gcol = t4 * TOK
                xts = []
                for cc in range(NCIN):
                    xt = xtp.tile([128, TOK], F16, tag="xt", name="xt")
                    nc.sync.dma_start(
                        xt[:], xT[cc * 128:(cc + 1) * 128, gcol:gcol + TOK])
                    xts.append(xt)
                return xts

            def qkv_chunk(t4, xts=None):
                gcol = t4 * TOK
                if xts is None:
                    xts = load_xts(t4)
                ps_r = ps.tile([128, TOK], F32, tag="acc", bufs=2)
                for cc in range(NCIN):
                    nc.tensor.matmul(ps_r[:], aq_sb[:, cc, :], xts[cc][:],
                                     start=(cc == 0), stop=(cc == NCIN - 1))
                r_sb = small.tile([128, TOK], F16, tag="r")
                nc.vector.tensor_copy(r_sb[:], ps_r[:])
                nc.vector.memset(r_sb[64:65, :], 1.0)  # bias row
                # chunk order: v (hp0..3), k, q  -> ch = ty*4+hp
                for ty, dest in ((2, v4), (1, k4), (0, q4)):
                    for hp in range(HPG):
                        ch = ty * 4 + hp
                        ps_q = ps.tile([128, TOK], F32, tag="acc", bufs=2)
                        for cc in range(NCIN):
                            nc.tensor.matmul(
                                ps_q[:],
                                wq_sb[:, cc, ch * 128:(ch + 1) * 128],
                                xts[cc][:], start=(cc == 0), stop=False)
                        nc.tensor.matmul(
                            ps_q[:], lb_sb[:, ch * 128:(ch + 1) * 128],
                            r_sb[:], start=False, stop=True)
                        dst = dest[:, hp, gcol:gcol + TOK]
                        if ch % 2 == 0:
                            nc.vector.tensor_copy(dst, ps_q[:])
                        else:
                            nc.scalar.copy(dst, ps_q[:])

            def vtr_chunk(t4):
                for tb in range(4 * t4, 4 * t4 + 4):
                    for hp in range(HPG):
                        ps_t = ps.tile([128, 128], F16, tag="qk", bufs=2)
                        nc.tensor.transpose(
                            ps_t[:], v4[:, hp, tb * 128:(tb + 1) * 128],
                            eye_sb[:])
                        nc.vector.tensor_copy(
                            vaA[hp][:, tb * 66:tb * 66 + 64], ps_t[:, 0:64])
                        nc.vector.tensor_copy(
                            vaB[hp][:, tb * 130 + 64:tb * 130 + 128],
                            ps_t[:, 64:128])

            def attn_chunk(t4, hp):
                nblk = 4 * (t4 + 1)
                tq = t4 * TOK
                psy0 = ps.tile([128, TOK], F32, tag="pv", bufs=2)
                psy1 = ps.tile([128, TOK], F32, tag="pv", bufs=2)
                q0s, exps = {}, {}

                def emit_qk(j):
                    r = j - 4 * t4
                    q0 = 128 * r if r > 0 else 0
                    q0s[j] = q0
                    s = ps.tile([128, 2, TOK], F32, tag="qk", bufs=2)
                    jc = slice(j * 128, (j + 1) * 128)
                    qc = slice(tq + q0, tq + TOK)
                    nc.tensor.matmul(s[:, 0, q0:TOK], k4[0:64, hp, jc],
                                     q4[0:64, hp, qc], start=True, stop=True)
                    nc.tensor.matmul(s[:, 1, q0:TOK], k4[64:128, hp, jc],
                                     q4[64:128, hp, qc], start=True,
                                     stop=True)
                    e = expp.tile([128, 2, TOK], F16, tag="e")
                    nc.scalar.activation(out=e[:, :, q0:TOK],
                                         in_=s[:, :, q0:TOK],
                                         func=EXPF, scale=0.125)
                    if r >= 0:
                        e2 = e[:, :, q0:q0 + 128]
                        trib = bass.broadcast_tensor_aps(
                            e2, tri_sb[:].unsqueeze(1))[1]
                        nc.vector.tensor_mul(e2, e2, trib)
                    if DBG and t4 == 0 and hp == 0:
                        nc.sync.dma_start(des[j][:], e[:])
                    exps[j] = e

                def emit_pv(j):
                    q0 = q0s[j]
                    e = exps.pop(j)
                    st, sp = (j == 0), (j == nblk - 1)
                    nc.tensor.matmul(psy0[:, q0:TOK],
                                     vaA[hp][:, j * 66:j * 66 + 128],
                                     e[:, 0, q0:TOK], start=st, stop=sp)
                    nc.tensor.matmul(psy1[:, q0:TOK],
                                     vaB[hp][:, j * 130:j * 130 + 128],
                                     e[:, 1, q0:TOK], start=st, stop=sp)

                LA = 2
                for jj in range(min(LA, nblk)):
                    emit_qk(jj)
                for j in range(nblk):
                    if j + LA < nblk:
                        emit_qk(j + LA)
                    emit_pv(j)

                tsl = slice(tq, tq + TOK)
                # h0 chain start: z at psum row 64 -> DMA hop to partition 0
                zc0 = small.tile([65, TOK], F32, tag="zc0")
                nc.vector.tensor_copy(zc0[64:65, :], psy0[64:65, :])
                z0 = small.tile([1, TOK], F32, tag="z0")
                nc.sync.dma_start(z0[:], zc0[64:65, :])
                # h1 full chain (no DMA dependency): frees psy1 quickly
                zc1 = small.tile([1, TOK], F32, tag="zc1")
                nc.vector.tensor_copy(zc1[:], psy1[0:1, :])
                r1 = small.tile([1, TOK], F32, tag="r1")
                nc.vector.reciprocal_approx_fast(out=r1[:], in_=zc1[:])
                sbb1 = small.tile([128, TOK], F32, tag="sbb1")
                nc.gpsimd.partition_broadcast(sbb1[:], r1[0:1, :])
                nc.vector.tensor_mul(y4[64:128, hp, tsl], psy1[64:128, :],
                                     sbb1[64:128, :])
                # h0 remainder
                r0 = small.tile([1, TOK], F32, tag="r0")
                nc.vector.reciprocal_approx_fast(out=r0[:], in_=z0[:])
                sbb0 = small.tile([64, TOK], F32, tag="sbb0")
                nc.gpsimd.partition_broadcast(sbb0[:], r0[0:1, :])
                nc.vector.tensor_mul(y4[0:64, hp, tsl], psy0[0:64, :],
                                     sbb0[:])

            def proj_chunk(t4, borrow=False):
                gcol = t4 * TOK
                tsl = slice(gcol, gcol + TOK)
                ps_rp = ps.tile([128, TOK], F32, tag="acc", bufs=2)
                for hp in range(HPG):
                    nc.tensor.matmul(ps_rp[:], ap_sb[:, hp, :],
                                     y4[:, hp, tsl], start=(hp == 0),
                                     stop=(hp == HPG - 1))
                rp_sb = small.tile([128, TOK], F16, tag="rp")
                nc.vector.tensor_copy(rp_sb[:], ps_rp[:])
                for co in range(NCIN):
                    if borrow and co % 2 == 1:
                        ps_o = ps.tile([128, TOK], F32, tag="qk", bufs=2)
                    else:
                        ps_o = ps.tile([128, TOK], F32, tag="acc", bufs=2)
                    for hp in range(HPG):
                        nc.tensor.matmul(
                            ps_o[:], wp_sb[:, hp, co * 128:(co + 1) * 128],
                            y4[:, hp, tsl], start=(hp == 0), stop=False)
                    nc.tensor.matmul(ps_o[:],
                                     pb_sb[:, co * 128:(co + 1) * 128],
                                     rp_sb[:], start=False, stop=True)
                    po = small.tile([128, TOK], F16, tag="po", bufs=4)
                    if t4 == 3:
                        nc.vector.tensor_copy(po[:, 0:256], ps_o[:, 0:256])
                        nc.scalar.copy(po[:, 256:TOK], ps_o[:, 256:TOK])
                    elif co % 2 == 0:
                        nc.vector.tensor_copy(po[:], ps_o[:])
                    else:
                        nc.scalar.copy(po[:], ps_o[:])
                    nc.sync.dma_start(
                        outT[co * 128:(co + 1) * 128, tsl], po[:])

            # schedule: stagger so heavy late-attention ACT work overlaps
            # proj PE work, and qkv PE work overlaps early attention ACT.
            xts0 = load_xts(0)
            for cc in range(NCIN):
                nc.sync.dma_start(wq_sb[:, cc, :], Wq[:, cc, :])
            with nc.named_scope("qkv0"):
                qkv_chunk(0, xts0)
            with nc.named_scope("vtr0"):
                vtr_chunk(0)
            nc.sync.dma_start(wp_sb[:], Wp[:])
            nc.sync.dma_start(ap_sb[:], Ap[:])
            nc.sync.dma_start(pb_sb[:], Bp[:])
            with nc.named_scope("qkv1"):
                qkv_chunk(1)
            with nc.named_scope("vtr1"):
                vtr_chunk(1)
            with nc.named_scope("attn0"):
                for hp in range(HPG):
                    attn_chunk(0, hp)
            with nc.named_scope("qkv2"):
                qkv_chunk(2)
            with nc.named_scope("vtr2"):
                vtr_chunk(2)
            with nc.named_scope("attn1"):
                for hp in range(HPG):
                    attn_chunk(1, hp)
            with nc.named_scope("qkv3"):
                qkv_chunk(3)
            with nc.named_scope("vtr3"):
                vtr_chunk(3)
            with nc.named_scope("attn2"):
                for hp in range(HPG):
                    attn_chunk(2, hp)
            with nc.named_scope("proj0"):
                proj_chunk(0)
            with nc.named_scope("attn3"):
                for hp in range(HPG):
                    attn_chunk(3, hp)
            with nc.named_scope("proj1"):
                proj_chunk(1)
            with nc.named_scope("proj2"):
                proj_chunk(2)
            with nc.named_scope("proj3"):
                proj_chunk(3, borrow=True)
            if DBG:
                nc.sync.dma_start(dq4[:], q4[:])
                nc.sync.dma_start(dk4[:], k4[:])
                nc.sync.dma_start(dv4[:], v4[:])
                nc.sync.dma_start(dy4[:], y4[:])
                nc.sync.dma_start(dvaA[:], vaA[0][:])
                nc.sync.dma_start(dvaB[:], vaB[0][:])
    nc.compile()
    return nc


def _prep_inputs(x, W_attn, b_attn, A_attn, B_attn, W_proj, b_proj, A_proj,
                 B_proj):
    h = np.float16
    xTfull = np.ascontiguousarray(
        x.reshape(B * T, C).T.astype(h))           # [C, B*T]
    AqT = np.zeros((C, 128), np.float32)
    AqT[:, :RANK] = A_attn.T
    Aq = np.ascontiguousarray(
        AqT.reshape(NCIN, 128, 128).transpose(1, 0, 2)).astype(h)
    tri_m = np.triu(np.ones((128, 128), np.float32)).astype(h)
    eye_m = np.eye(128, dtype=np.float32).astype(h)
    ones_m = np.ones((128, 16), h)
    Bp_s = np.zeros((128, C), np.float32)
    Bp_s[:RANK] = (B_proj * SCALING).T
    Bp_s = Bp_s.astype(h)
    in_maps = []
    for c in range(NCORES):
        b, g = c // 2, c % 2
        rows = np.r_[g * 512:(g + 1) * 512,
                     C + g * 512:C + (g + 1) * 512,
                     2 * C + g * 512:2 * C + (g + 1) * 512]
        # chunk ch = ty*4+hp covers rows [ty*512 + hp*128 : +128] of `rows`
        W_sl = W_attn[rows]                                  # [1536, C]
        WqT = np.ascontiguousarray(
            W_sl.T.reshape(NCIN, 128, NCH * 128).transpose(1, 0, 2)
        ).astype(h)
        Bq_s = np.zeros((128, NCH * 128), np.float32)
        Bq_s[:RANK] = (B_attn[rows] * SCALING).T
        Bq_s[64] = b_attn[rows]                              # bias row
        ysl = slice(g * 512, (g + 1) * 512)
        WpT = np.ascontiguousarray(
            W_proj[:, ysl].T.reshape(HPG, 128, C).transpose(1, 0, 2)
        ).astype(h)                                          # [128, 4, C]
        ApT = np.zeros((512, 128), np.float32)
        ApT[:, :RANK] = A_proj[:, ysl].T
        ApT = np.ascontiguousarray(
            ApT.reshape(HPG, 128, 128).transpose(1, 0, 2)).astype(h)
        in_maps.append({
            "xT": np.ascontiguousarray(xTfull[:, b * T:(b + 1) * T]),
            "Wq": WqT, "Aq": Aq, "Bq": Bq_s.astype(h),
            "Wp": WpT, "Ap": ApT, "Bp": Bp_s,
            "tri": tri_m, "onesb": ones_m, "eye": eye_m,
        })
    return in_maps


def _install_ntff_shim():
    """Provide antenv.axon_hooks (missing on this image) via ctypes against
    the axon .so, mirroring trn_agent_boot.trn_boot._ntff_profile_via_ctypes."""
    import types
    import ctypes
    import contextlib
    try:
        from antenv.axon_hooks import get_axon_ntff_profile_hook  # noqa: F401
        return
    except ImportError:
        pass
    so_path = "/opt/axon/libaxon_pjrt.so"
    try:
        lib = ctypes.CDLL(so_path)
    except OSError:
        return
    if not hasattr(lib, "axon_start_nrt_profile"):
        return
    lib.axon_start_nrt_profile.argtypes = [ctypes.POINTER(ctypes.c_int64),
                                           ctypes.c_size_t]
    lib.axon_start_nrt_profile.restype = ctypes.c_int64
    lib.axon_stop_nrt_profile.argtypes = [ctypes.c_char_p]
    lib.axon_stop_nrt_profile.restype = ctypes.c_int64

    @contextlib.contextmanager
    def _hook(output_dir, device_ids):
        import jax
        jax.devices()
        if device_ids:
            ids = (ctypes.c_int64 * len(device_ids))(*device_ids)
            rc = lib.axon_start_nrt_profile(ids, len(device_ids))
        else:
            rc = lib.axon_start_nrt_profile(None, 0)
        if rc != 0:
            raise RuntimeError(f"axon_start_nrt_profile rc={rc}")
        try:
            yield
        finally:
            n = lib.axon_stop_nrt_profile(str(output_dir).encode())
            if n < 0:
                raise RuntimeError(f"axon_stop_nrt_profile rc={n}")

    import antenv
    mod = types.ModuleType("antenv.axon_hooks")
    mod.get_axon_ntff_profile_hook = lambda: _hook
    mod.set_axon_ntff_profile_hook = lambda h: None
    sys.modules["antenv.axon_hooks"] = mod
    antenv.axon_hooks = mod


def run(inputs, trace=False, trace_cores=None):
    """Run the kernel. Returns (output, BassKernelResults)."""
    if "nc" not in _cache:
        _cache["nc"] = _build()
    nc = _cache["nc"]
    inputs = {k: np.asarray(v, dtype=np.float32) for k, v in inputs.items()}
    in_maps = _prep_inputs(**inputs)
    if trace:
        _install_ntff_shim()
    res = run_bass_kernel_spmd(nc, in_maps, core_ids=list(range(NCORES)),
                               trace=trace, trace_cores=trace_cores)
    out = np.empty((B, T, C), np.float32)
    for b in range(B):
        oT = (res.results[2 * b]["outT"].astype(np.float32)
              + res.results[2 * b + 1]["outT"].astype(np.float32))
        out[b] = oT.T + inputs["b_proj"][None, :]
    return out, res


def kernel(**inputs):
    out, _ = run(inputs, trace=False)
    return out

